# revision 6
# baseline (speedup 1.0000x reference)
"""Trainium2 Bass kernel for nn_RelativeMultiHeadAttention.

Full (unsharded) numpy inputs in, full output out. Internally shards across
8 NeuronCores: core c handles batch b = c//4 and head pair hp = c%4
(heads 2*hp, 2*hp+1).

Single software-pipelined device loop (per core):
  - projections on PE produce q_T/k_T/p_T ([2*dh, L], channels on partitions)
    and v ([L, 2*dh], natural), biases folded in via ACT bias adds.
  - pos scores computed NATURAL ([q, k]) per 128-row q tile, cast to fp8 and
    streamed to a DRAM buffer G with rows padded to L+1 (pad col = 0).
  - relative shift: shifted[q, k] = G_flat[q*L + (L-1) + k]  (Transformer-XL
    pad/reshape/slice trick as a strided read of the padded flat buffer).
    Read back NATURAL ([q, k], fully contiguous DMA, fp8).
  - content scores computed TRANSPOSED ([k, q]) into PSUM; the shifted tiles
    are transposed on the PE (matmul against identity) and ACCUMULATED into
    the same PSUM bank, so the add costs no DVE time and no extra traffic.
  - ACT applies exp(scale * logit) straight from PSUM -> attn_T (unnormalized
    bf16; |scale*logit| < ~4 so no max-subtraction needed).
  - A.V per 512-wide q chunk: lhsT = [v_h | ones] so PSUM row 64 accumulates
    Z = sum_k attn. Out projection per head (K=64) + per-partition 1/Z
    normalization, summed across the two heads on DVE; partial out bf16.
Host sums the 4 per-core partials of each batch and adds bv @ Wo + bo
(exact: attention rows sum to 1 after normalization).
"""

import numpy as np
import ml_dtypes

B, L, D, H = 2, 2048, 512, 8
DH = D // H            # 64
HPC = 2                # heads per core
NCORES = 8
SCALE = 1.0 / float(np.sqrt(D))
NQT = L // 128         # 16 q/k tiles of 128
NQC = L // 512         # 4 chunks of 512
TPC = NQT // NQC       # 4 q-tiles per chunk
GROWS = L + 1          # padded G row length (2049)

_BF16 = ml_dtypes.bfloat16

# G (pos-score scratch) dtype: fp8e4 halves the DRAM roundtrip vs bf16.
G_FP8 = True


def build_nc():
    import concourse.bass as bass
    import concourse.mybir as mybir
    from concourse.bacc import Bacc
    from concourse.tile import TileContext
    from contextlib import ExitStack

    bf16 = mybir.dt.bfloat16
    f32 = mybir.dt.float32
    gdt = mybir.dt.float8e4 if G_FP8 else bf16
    AF = mybir.ActivationFunctionType
    ALU = mybir.AluOpType

    nc = Bacc()

    # ---- I/O ----
    xq = nc.declare_dram_parameter("xq_t", [D, L], bf16, isOutput=False)
    xk = nc.declare_dram_parameter("xk_t", [D, L], bf16, isOutput=False)
    xp = nc.declare_dram_parameter("xp_t", [D, L], bf16, isOutput=False)
    xv = nc.declare_dram_parameter("xv_t", [D, L], bf16, isOutput=False)
    wq = nc.declare_dram_parameter("wq", [D, HPC * DH], bf16, isOutput=False)
    wk = nc.declare_dram_parameter("wk", [D, HPC * DH], bf16, isOutput=False)
    wp = nc.declare_dram_parameter("wp", [D, HPC * DH], bf16, isOutput=False)
    wv = nc.declare_dram_parameter("wv", [D, HPC * DH], bf16, isOutput=False)
    wo0 = nc.declare_dram_parameter("wo0", [DH, D], bf16, isOutput=False)
    wo1 = nc.declare_dram_parameter("wo1", [DH, D], bf16, isOutput=False)
    ident = nc.declare_dram_parameter("ident", [128, 128], bf16, isOutput=False)
    bq = nc.declare_dram_parameter("bq", [HPC * DH, 1], f32, isOutput=False)
    bk = nc.declare_dram_parameter("bk", [HPC * DH, 1], f32, isOutput=False)
    out = nc.declare_dram_parameter("out", [L, D], bf16, isOutput=True)

    # scratch DRAM for the relative-shift roundtrip, one per head
    g0 = nc.dram_tensor("g0", [L * GROWS], gdt)
    g1 = nc.dram_tensor("g1", [L * GROWS], gdt)
    gs = [g0, g1]

    with TileContext(nc) as tc, ExitStack() as top:
        # ---------- persistent SBUF ----------
        persist = top.enter_context(tc.tile_pool(name="persist", bufs=1))

        def ptile(shape, dtype, name):
            return persist.tile(shape, dtype, name=name, tag=name)

        qT = ptile([128, L], bf16, "qT")
        kT = ptile([128, L], bf16, "kT")
        pT = ptile([128, L], bf16, "pT")
        vaug = ptile([128, NQT, HPC * (DH + 1)], bf16, "vaug")
        wo_sb0 = ptile([DH, D], bf16, "wo_sb0")
        wo_sb1 = ptile([DH, D], bf16, "wo_sb1")
        bq_sb = ptile([128, 1], f32, "bq_sb")
        bk_sb = ptile([128, 1], f32, "bk_sb")
        ones_sb = ptile([1, 1], f32, "ones_sb")
        id_sb = ptile([128, 128], bf16, "id_sb")

        nc.vector.memset(ones_sb[:, :], 1.0)
        nc.vector.memset(vaug[:, :, DH:DH + 1], 1.0)
        nc.vector.memset(vaug[:, :, 2 * DH + 1:2 * DH + 2], 1.0)
        nc.gpsimd.dma_start(out=id_sb[:, :], in_=ident[:, :])
        nc.gpsimd.dma_start(out=wo_sb0[:, :], in_=wo0[:, :])
        nc.gpsimd.dma_start(out=wo_sb1[:, :], in_=wo1[:, :])
        nc.gpsimd.dma_start(out=bq_sb[:, :], in_=bq[:, :])
        nc.gpsimd.dma_start(out=bk_sb[:, :], in_=bk[:, :])

        # ---------- phase 1: projections ----------
        with ExitStack() as p1:
            inpool = p1.enter_context(tc.tile_pool(name="inpool", bufs=1))
            x_sbs = {}
            w_sbs = {}
            # load q,p first (pos scores need them), k,v later
            for name, src, eng in (("q", xq, nc.sync), ("p", xp, nc.sync),
                                   ("k", xk, nc.scalar), ("v", xv, nc.scalar)):
                t = inpool.tile([128, 4, L], bf16, name=f"x_{name}", tag=f"x_{name}")
                for c in range(4):
                    eng.dma_start(out=t[:, c, :], in_=src[c * 128:(c + 1) * 128, :])
                x_sbs[name] = t
            for name, src in (("q", wq), ("p", wp), ("k", wk), ("v", wv)):
                t = inpool.tile([128, 4, HPC * DH], bf16, name=f"w_{name}",
                                tag=f"w_{name}")
                nc.gpsimd.dma_start(
                    out=t[:, :, :], in_=src[:, :].rearrange("(c p) m -> p c m", p=128))
                w_sbs[name] = t

            pj_psum = p1.enter_context(
                tc.tile_pool(name="pj_psum", bufs=2, space="PSUM"))

            # q_T / p_T / k_T : [128 (2 heads * 64 ch), L]
            for name, dst, bias in (("q", qT, bq_sb), ("p", pT, None),
                                    ("k", kT, bk_sb)):
                xs, ws = x_sbs[name], w_sbs[name]
                for n in range(NQC):
                    ps = pj_psum.tile([128, 512], f32, tag="pj")
                    for c in range(4):
                        nc.tensor.matmul(
                            ps[:, :], lhsT=ws[:, c, :],
                            rhs=xs[:, c, n * 512:(n + 1) * 512],
                            start=(c == 0), stop=(c == 3))
                    o = dst[:, n * 512:(n + 1) * 512]
                    if bias is not None:
                        nc.scalar.activation(o, ps[:, :], AF.Identity,
                                             bias=bias[:, 0:1], scale=1.0)
                    else:
                        nc.scalar.copy(o, ps[:, :])

            # v natural: [L, 128ch] -> vaug [128, t, [v0|1|v1|1]]
            xs, ws = x_sbs["v"], w_sbs["v"]
            for t in range(NQT):
                ps = pj_psum.tile([128, 128], f32, tag="pj")
                for c in range(4):
                    nc.tensor.matmul(
                        ps[:, :], lhsT=xs[:, c, t * 128:(t + 1) * 128],
                        rhs=ws[:, c, :], start=(c == 0), stop=(c == 3))
                nc.vector.tensor_copy(vaug[:, t, 0:DH], ps[:, 0:DH])
                nc.vector.tensor_copy(vaug[:, t, DH + 1:2 * DH + 1],
                                      ps[:, DH:2 * DH])

        # ---------- phase 2: fused pipelined main loop ----------
        pos_psum = top.enter_context(
            tc.tile_pool(name="pos_psum", bufs=2, space="PSUM"))
        ct_psum = top.enter_context(
            tc.tile_pool(name="ct_psum", bufs=2, space="PSUM"))
        cx_psum = top.enter_context(
            tc.tile_pool(name="cx_psum", bufs=2, space="PSUM"))
        po_psum = top.enter_context(
            tc.tile_pool(name="po_psum", bufs=2, space="PSUM"))
        st_pool = top.enter_context(tc.tile_pool(name="st_pool", bufs=4))
        sh_pool = top.enter_context(tc.tile_pool(name="sh_pool", bufs=2))
        attn_pool = top.enter_context(tc.tile_pool(name="attn_pool", bufs=2))
        ctx_pool = top.enter_context(tc.tile_pool(name="ctx_pool", bufs=2))
        z_pool = top.enter_context(tc.tile_pool(name="z_pool", bufs=2))
        rz_pool = top.enter_context(tc.tile_pool(name="rz_pool", bufs=2))
        tmp_pool = top.enter_context(tc.tile_pool(name="tmp_pool", bufs=2))
        out_pool = top.enter_context(tc.tile_pool(name="out_pool", bufs=3))

        # pos-score production is split into (tile, nk) quanta so it can be
        # interleaved into the content loop: one quantum = paired h0/h1
        # matmuls [128q, 512k] + psum->fp8 casts (DVE for h0, ACT for h1).
        pos_state = {}

        def pos_quantum(t, nk):
            if nk == 0:
                sts = []
                for h in range(HPC):
                    st = st_pool.tile([128, GROWS], gdt, tag=f"st{h}",
                                      name=f"st{h}")
                    nc.vector.memset(st[:, L:GROWS], 0.0)
                    sts.append(st)
                pos_state[t] = sts
            sts = pos_state[t]
            pss = []
            for h in range(HPC):
                hb = h * DH
                ps = pos_psum.tile([128, 512], f32, tag="pos", name="ps_pos")
                nc.tensor.matmul(
                    ps[:, :],
                    lhsT=qT[hb:hb + DH, t * 128:(t + 1) * 128],
                    rhs=pT[hb:hb + DH, nk * 512:(nk + 1) * 512],
                    start=True, stop=True)
                pss.append(ps)
            nc.vector.tensor_copy(sts[0][:, nk * 512:(nk + 1) * 512],
                                  pss[0][:, :])
            nc.scalar.copy(sts[1][:, nk * 512:(nk + 1) * 512], pss[1][:, :])
            if nk == NQC - 1:
                for h in range(HPC):
                    nc.gpsimd.dma_start(
                        out=bass.AP(gs[h], t * 128 * GROWS,
                                    [[GROWS, 128], [1, GROWS]]),
                        in_=sts[h][:, :])
                del pos_state[t]

        def emit_chunk(c, pos_tiles):
            """shifted reads + content+shift-add+exp + A.V + out-proj for
            q chunk c (512 q rows). pos_tiles: next-chunk pos-score tiles
            to interleave into the 16 content kt slots."""
            q0 = c * 512
            quanta = [(t, nk) for t in pos_tiles for nk in range(NQC)]
            # shifted natural reads: [128q, L k] fp8, contiguous in G.
            # Row 127 of tile qt depends on the NEXT pos tile's G write, so
            # it is read separately to keep the main read's dep one tile
            # earlier (precise AP-overlap tracking does the rest).
            shs = {}
            for tl in range(TPC):
                qt = c * TPC + tl
                for h in range(HPC):
                    sh = sh_pool.tile([128, L], gdt, tag=f"sh{h}_{tl}",
                                      name=f"sh{h}_{tl}")
                    nc.sync.dma_start(
                        out=sh[0:127, :],
                        in_=bass.AP(gs[h], (L - 1) + qt * 128 * L,
                                    [[L, 127], [1, L]]))
                    nc.sync.dma_start(
                        out=sh[127:128, :],
                        in_=bass.AP(gs[h], (L - 1) + (qt * 128 + 127) * L,
                                    [[L, 1], [1, L]]))
                    shs[(tl, h)] = sh

            # content transposed + shifted-transpose-accumulate + exp
            attns = []
            for h in range(HPC):
                at = attn_pool.tile([128, NQT, 512], bf16, tag=f"attn{h}",
                                    name=f"attn{h}")
                attns.append(at)
            for kt in range(NQT):
                if kt < len(quanta):
                    pos_quantum(*quanta[kt])
                pcs = []
                for h in range(HPC):
                    hb = h * DH
                    pc = ct_psum.tile([128, 512], f32, tag="ct", name="ct")
                    nc.tensor.matmul(
                        pc[:, :],
                        lhsT=kT[hb:hb + DH, kt * 128:(kt + 1) * 128],
                        rhs=qT[hb:hb + DH, q0:q0 + 512],
                        start=True, stop=False)
                    pcs.append(pc)
                for h in range(HPC):
                    for tl in range(TPC):
                        nc.tensor.matmul(
                            pcs[h][:, tl * 128:(tl + 1) * 128],
                            lhsT=shs[(tl, h)][:, kt * 128:(kt + 1) * 128],
                            rhs=id_sb[:, :],
                            start=False, stop=(tl == TPC - 1))
                for h in range(HPC):
                    nc.scalar.activation(
                        attns[h][:, kt, :], pcs[h][:, :],
                        AF.Exp, bias=0.0, scale=SCALE)

            # A.V (transposed): ctx_T [64, 512] + Z row via ones-col.
            # cx is a full [128, 512] bank; after it drains, the Z-row
            # transpose matmuls (Z values onto partitions) reuse it.
            ctxs = []
            zrows = []
            cxs = []
            for h in range(HPC):
                cx = cx_psum.tile([128, 512], f32, tag="cx", name="cx")
                for kt in range(NQT):
                    nc.tensor.matmul(
                        cx[0:DH + 1, :],
                        lhsT=vaug[:, kt, h * (DH + 1):(h + 1) * (DH + 1)],
                        rhs=attns[h][:, kt, :],
                        start=(kt == 0), stop=(kt == NQT - 1))
                ctx = ctx_pool.tile([DH, 512], bf16, tag=f"ctx{h}",
                                    name=f"ctx{h}")
                zrow = z_pool.tile([1, 512], f32, tag=f"z{h}", name=f"z{h}")
                nc.vector.tensor_copy(ctx[:, :], cx[0:DH, :])
                nc.scalar.copy(zrow[0:1, :], cx[DH:DH + 1, :])
                ctxs.append(ctx)
                zrows.append(zrow)
                cxs.append(cx)
            rzs = []
            for h in range(HPC):
                for tl in range(TPC):
                    nc.tensor.matmul(
                        cxs[h][:, tl:tl + 1],
                        lhsT=zrows[h][0:1, tl * 128:(tl + 1) * 128],
                        rhs=ones_sb[0:1, 0:1], start=True, stop=True)
                rz = rz_pool.tile([128, TPC], f32, tag=f"rz{h}",
                                  name=f"rz{h}")
                nc.vector.reciprocal(rz[:, :], cxs[h][:, 0:TPC])
                rzs.append(rz)

            # out projection + 1/Z normalization per 128-row tile
            for tl in range(TPC):
                po0 = po_psum.tile([128, 512], f32, tag="po")
                nc.tensor.matmul(po0[:, :],
                                 lhsT=ctxs[0][:, tl * 128:(tl + 1) * 128],
                                 rhs=wo_sb0[:, :], start=True, stop=True)
                po1 = po_psum.tile([128, 512], f32, tag="po")
                nc.tensor.matmul(po1[:, :],
                                 lhsT=ctxs[1][:, tl * 128:(tl + 1) * 128],
                                 rhs=wo_sb1[:, :], start=True, stop=True)
                tm = tmp_pool.tile([128, 512], f32, tag="tmp")
                nc.vector.tensor_scalar_mul(tm[:, :], po0[:, :],
                                            rzs[0][:, tl:tl + 1])
                ot = out_pool.tile([128, 512], bf16, tag="out")
                nc.vector.scalar_tensor_tensor(
                    ot[:, :], po1[:, :], rzs[1][:, tl:tl + 1], tm[:, :],
                    op0=ALU.mult, op1=ALU.add)
                trow = c * TPC + tl
                nc.scalar.dma_start(out=out[trow * 128:(trow + 1) * 128, :],
                                    in_=ot[:, :])

        # software pipeline: pos runs one chunk (+1 boundary tile) ahead of
        # the content/attention chunk, so the G write->read roundtrip
        # overlaps. Each chunk's next-next boundary tile goes first in its
        # interleave list so its G write lands before the next chunk needs
        # its row-127 reads.
        for t in range(TPC + 1):
            for nk in range(NQC):
                pos_quantum(t, nk)
        interleave = [[2 * TPC], [3 * TPC], [], []]
        for c in range(2):
            interleave[c] += list(range((c + 1) * TPC + 1, (c + 2) * TPC))
        interleave[2] = list(range(3 * TPC + 1, NQT))
        for c in range(NQC):
            emit_chunk(c, interleave[c])

    return nc


def _shard_inputs(query, key, value, pos_emb, Wq, bq, Wk, bk, Wv, bv, Wp, Wo, bo):
    """Build the 8 per-core input maps (host-side, free)."""
    in_maps = []
    xt = {}
    for b in range(B):
        xt[("q", b)] = np.ascontiguousarray(query[b].T).astype(_BF16)
        xt[("k", b)] = np.ascontiguousarray(key[b].T).astype(_BF16)
        xt[("p", b)] = np.ascontiguousarray(pos_emb[b].T).astype(_BF16)
        xt[("v", b)] = np.ascontiguousarray(value[b].T).astype(_BF16)
    wq16, wk16, wp16, wv16, wo16 = (w.astype(_BF16) for w in (Wq, Wk, Wp, Wv, Wo))
    ident = np.eye(128, dtype=np.float32).astype(_BF16)
    for c in range(NCORES):
        b, hp = c // 4, c % 4
        cs = slice(hp * HPC * DH, (hp + 1) * HPC * DH)
        in_maps.append({
            "xq_t": xt[("q", b)],
            "xk_t": xt[("k", b)],
            "xp_t": xt[("p", b)],
            "xv_t": xt[("v", b)],
            "ident": ident,
            "wq": np.ascontiguousarray(wq16[:, cs]),
            "wk": np.ascontiguousarray(wk16[:, cs]),
            "wp": np.ascontiguousarray(wp16[:, cs]),
            "wv": np.ascontiguousarray(wv16[:, cs]),
            "wo0": np.ascontiguousarray(wo16[hp * HPC * DH:hp * HPC * DH + DH, :]),
            "wo1": np.ascontiguousarray(wo16[hp * HPC * DH + DH:(hp + 1) * HPC * DH, :]),
            "bq": np.ascontiguousarray(bq[cs]).reshape(HPC * DH, 1).astype(np.float32),
            "bk": np.ascontiguousarray(bk[cs]).reshape(HPC * DH, 1).astype(np.float32),
        })
    return in_maps


def _unshard(results, Wo, bv, bo):
    const = (bv.astype(np.float32) @ Wo.astype(np.float32)) + bo.astype(np.float32)
    out = np.zeros((B, L, D), np.float32)
    for c in range(NCORES):
        out[c // 4] += results[c]["out"].astype(np.float32)
    out += const[None, None, :]
    return out


_CACHE = {}


def kernel(query, key, value, pos_emb, Wq, bq, Wk, bk, Wv, bv, Wp, Wo, bo,
           _want_profile=False):
    import sys
    if "/opt/trn_rl_repo" not in sys.path:
        sys.path.insert(0, "/opt/trn_rl_repo")
    from concourse.bass_utils import run_bass_kernel_spmd

    args = [np.asarray(a) for a in
            (query, key, value, pos_emb, Wq, bq, Wk, bk, Wv, bv, Wp, Wo, bo)]
    (query, key, value, pos_emb, Wq, bq, Wk, bk, Wv, bv, Wp, Wo, bo) = args

    if "nc" not in _CACHE:
        nc = build_nc()
        if not nc.is_finalized():
            nc.finalize()
        _CACHE["nc"] = nc
    nc = _CACHE["nc"]

    in_maps = _shard_inputs(query, key, value, pos_emb, Wq, bq, Wk, bk, Wv, bv,
                            Wp, Wo, bo)
    res = run_bass_kernel_spmd(nc, in_maps, list(range(NCORES)),
                               trace=_want_profile)
    out = _unshard(res.results, Wo, bv, bo)
    if _want_profile:
        return out, res
    return out


# revision 9
# speedup vs baseline: 1.4945x; 1.4945x over previous
"""Trainium2 Bass kernel for nn_RelativeMultiHeadAttention.

Full (unsharded) numpy inputs in, full output out. Internally shards across
8 NeuronCores: core c handles batch b = c//4 and head pair hp = c%4
(heads 2*hp, 2*hp+1).

Single software-pipelined device loop (per core):
  - projections on PE produce q_T/k_T/p_T ([2*dh, L], channels on partitions)
    and v ([L, 2*dh], natural), biases folded in via ACT bias adds.
  - pos scores computed NATURAL ([q, k]) per 128-row q tile, cast to fp8 and
    streamed to a DRAM buffer G with rows padded to L+1 (pad col = 0).
  - relative shift: shifted[q, k] = G_flat[q*L + (L-1) + k]  (Transformer-XL
    pad/reshape/slice trick as a strided read of the padded flat buffer).
    Read back NATURAL ([q, k], fully contiguous DMA, fp8).
  - content scores computed TRANSPOSED ([k, q]) into PSUM; the shifted tiles
    are transposed on the PE (matmul against identity) and ACCUMULATED into
    the same PSUM bank, so the add costs no DVE time and no extra traffic.
  - ACT applies exp(scale * logit) straight from PSUM -> attn_T (unnormalized
    bf16; |scale*logit| < ~4 so no max-subtraction needed).
  - A.V per 512-wide q chunk: lhsT = [v_h | ones] so PSUM row 64 accumulates
    Z = sum_k attn. Out projection per head (K=64) + per-partition 1/Z
    normalization, summed across the two heads on DVE; partial out bf16.
Host sums the 4 per-core partials of each batch and adds bv @ Wo + bo
(exact: attention rows sum to 1 after normalization).
"""

import numpy as np
import ml_dtypes

B, L, D, H = 2, 2048, 512, 8
DH = D // H            # 64
HPC = 2                # heads per core
NCORES = 8
SCALE = 1.0 / float(np.sqrt(D))
NQT = L // 128         # 16 q/k tiles of 128
NQC = L // 512         # 4 chunks of 512
TPC = NQT // NQC       # 4 q-tiles per chunk
GROWS = L + 1          # padded G row length (2049)

_BF16 = ml_dtypes.bfloat16

# G (pos-score scratch) dtype. fp8 would halve the DRAM roundtrip, but its
# 2048B-per-partition descriptor runs fall below the 4KB M2S-concat
# threshold (~60 GB/s measured vs ~350 for bf16's 4KB runs), and it eats
# most of the 2e-2 error budget. bf16 it is.
G_FP8 = False


def build_nc():
    import concourse.bass as bass
    import concourse.mybir as mybir
    from concourse.bacc import Bacc
    from concourse.tile import TileContext
    from contextlib import ExitStack

    bf16 = mybir.dt.bfloat16
    f32 = mybir.dt.float32
    gdt = mybir.dt.float8e4 if G_FP8 else bf16
    AF = mybir.ActivationFunctionType
    ALU = mybir.AluOpType

    nc = Bacc()

    # ---- I/O ----
    xq = nc.declare_dram_parameter("xq_t", [D, L], bf16, isOutput=False)
    xk = nc.declare_dram_parameter("xk_t", [D, L], bf16, isOutput=False)
    xp = nc.declare_dram_parameter("xp_t", [D, L], bf16, isOutput=False)
    xv = nc.declare_dram_parameter("xv_t", [D, L], bf16, isOutput=False)
    wq = nc.declare_dram_parameter("wq", [D, HPC * DH], bf16, isOutput=False)
    wk = nc.declare_dram_parameter("wk", [D, HPC * DH], bf16, isOutput=False)
    wp = nc.declare_dram_parameter("wp", [D, HPC * DH], bf16, isOutput=False)
    wv = nc.declare_dram_parameter("wv", [D, HPC * DH], bf16, isOutput=False)
    wo0 = nc.declare_dram_parameter("wo0", [DH, D], bf16, isOutput=False)
    wo1 = nc.declare_dram_parameter("wo1", [DH, D], bf16, isOutput=False)
    ident = nc.declare_dram_parameter("ident", [128, 128], bf16, isOutput=False)
    bq = nc.declare_dram_parameter("bq", [HPC * DH, 1], f32, isOutput=False)
    bk = nc.declare_dram_parameter("bk", [HPC * DH, 1], f32, isOutput=False)
    out = nc.declare_dram_parameter("out", [L, D], bf16, isOutput=True)

    # scratch DRAM for the relative-shift roundtrip, one per head
    g0 = nc.dram_tensor("g0", [L * GROWS], gdt)
    g1 = nc.dram_tensor("g1", [L * GROWS], gdt)
    gs = [g0, g1]

    with TileContext(nc) as tc, ExitStack() as top:
        # ---------- persistent SBUF ----------
        persist = top.enter_context(tc.tile_pool(name="persist", bufs=1))

        def ptile(shape, dtype, name):
            return persist.tile(shape, dtype, name=name, tag=name)

        qT = ptile([128, L], bf16, "qT")
        kT = ptile([128, L], bf16, "kT")
        pT = ptile([128, L], bf16, "pT")
        vaug = ptile([128, NQT, HPC * (DH + 1)], bf16, "vaug")
        wo_sb0 = ptile([DH, D], bf16, "wo_sb0")
        wo_sb1 = ptile([DH, D], bf16, "wo_sb1")
        bq_sb = ptile([128, 1], f32, "bq_sb")
        bk_sb = ptile([128, 1], f32, "bk_sb")
        ones_sb = ptile([1, 1], f32, "ones_sb")
        id_sb = ptile([128, 128], bf16, "id_sb")

        nc.vector.memset(ones_sb[:, :], 1.0)
        nc.vector.memset(vaug[:, :, DH:DH + 1], 1.0)
        nc.vector.memset(vaug[:, :, 2 * DH + 1:2 * DH + 2], 1.0)
        nc.gpsimd.dma_start(out=id_sb[:, :], in_=ident[:, :])
        nc.gpsimd.dma_start(out=wo_sb0[:, :], in_=wo0[:, :])
        nc.gpsimd.dma_start(out=wo_sb1[:, :], in_=wo1[:, :])
        nc.gpsimd.dma_start(out=bq_sb[:, :], in_=bq[:, :])
        nc.gpsimd.dma_start(out=bk_sb[:, :], in_=bk[:, :])

        # ---------- phase 1: projections ----------
        with ExitStack() as p1:
            inpool = p1.enter_context(tc.tile_pool(name="inpool", bufs=1))
            x_sbs = {}
            w_sbs = {}
            # load q,p first (pos scores need them), k,v later
            for name, src, eng in (("q", xq, nc.sync), ("p", xp, nc.sync),
                                   ("k", xk, nc.scalar), ("v", xv, nc.scalar)):
                t = inpool.tile([128, 4, L], bf16, name=f"x_{name}", tag=f"x_{name}")
                for c in range(4):
                    eng.dma_start(out=t[:, c, :], in_=src[c * 128:(c + 1) * 128, :])
                x_sbs[name] = t
            for name, src in (("q", wq), ("p", wp), ("k", wk), ("v", wv)):
                t = inpool.tile([128, 4, HPC * DH], bf16, name=f"w_{name}",
                                tag=f"w_{name}")
                nc.gpsimd.dma_start(
                    out=t[:, :, :], in_=src[:, :].rearrange("(c p) m -> p c m", p=128))
                w_sbs[name] = t

            pj_psum = p1.enter_context(
                tc.tile_pool(name="pj_psum", bufs=2, space="PSUM"))

            # q_T / p_T / k_T : [128 (2 heads * 64 ch), L]
            for name, dst, bias in (("q", qT, bq_sb), ("p", pT, None),
                                    ("k", kT, bk_sb)):
                xs, ws = x_sbs[name], w_sbs[name]
                for n in range(NQC):
                    ps = pj_psum.tile([128, 512], f32, tag="pj")
                    for c in range(4):
                        nc.tensor.matmul(
                            ps[:, :], lhsT=ws[:, c, :],
                            rhs=xs[:, c, n * 512:(n + 1) * 512],
                            start=(c == 0), stop=(c == 3))
                    o = dst[:, n * 512:(n + 1) * 512]
                    if bias is not None:
                        nc.scalar.activation(o, ps[:, :], AF.Identity,
                                             bias=bias[:, 0:1], scale=1.0)
                    else:
                        nc.scalar.copy(o, ps[:, :])

            # v natural: [L, 128ch] -> vaug [128, t, [v0|1|v1|1]]
            xs, ws = x_sbs["v"], w_sbs["v"]
            for t in range(NQT):
                ps = pj_psum.tile([128, 128], f32, tag="pj")
                for c in range(4):
                    nc.tensor.matmul(
                        ps[:, :], lhsT=xs[:, c, t * 128:(t + 1) * 128],
                        rhs=ws[:, c, :], start=(c == 0), stop=(c == 3))
                nc.vector.tensor_copy(vaug[:, t, 0:DH], ps[:, 0:DH])
                nc.vector.tensor_copy(vaug[:, t, DH + 1:2 * DH + 1],
                                      ps[:, DH:2 * DH])

        # ---------- phase 2: fused pipelined main loop ----------
        pos_psum = top.enter_context(
            tc.tile_pool(name="pos_psum", bufs=2, space="PSUM"))
        ct_psum = top.enter_context(
            tc.tile_pool(name="ct_psum", bufs=2, space="PSUM"))
        cx_psum = top.enter_context(
            tc.tile_pool(name="cx_psum", bufs=2, space="PSUM"))
        po_psum = top.enter_context(
            tc.tile_pool(name="po_psum", bufs=2, space="PSUM"))
        st_pool = top.enter_context(tc.tile_pool(name="st_pool", bufs=3))
        sh_pool = top.enter_context(tc.tile_pool(name="sh_pool", bufs=2))
        attn_pool = top.enter_context(tc.tile_pool(name="attn_pool", bufs=1))
        ctx_pool = top.enter_context(tc.tile_pool(name="ctx_pool", bufs=2))
        z_pool = top.enter_context(tc.tile_pool(name="z_pool", bufs=2))
        rz_pool = top.enter_context(tc.tile_pool(name="rz_pool", bufs=2))
        tmp_pool = top.enter_context(tc.tile_pool(name="tmp_pool", bufs=2))
        out_pool = top.enter_context(tc.tile_pool(name="out_pool", bufs=3))

        # pos-score production is split into (tile, nk) quanta so it can be
        # interleaved into the content loop: one quantum = paired h0/h1
        # matmuls [128q, 512k] + psum->fp8 casts (DVE for h0, ACT for h1).
        pos_state = {}

        def pos_quantum(t, nk):
            if nk == 0:
                sts = []
                for h in range(HPC):
                    st = st_pool.tile([128, GROWS], gdt, tag=f"st{h}",
                                      name=f"st{h}")
                    nc.vector.memset(st[:, L:GROWS], 0.0)
                    sts.append(st)
                pos_state[t] = sts
            sts = pos_state[t]
            pss = []
            for h in range(HPC):
                hb = h * DH
                ps = pos_psum.tile([128, 512], f32, tag="pos", name="ps_pos")
                nc.tensor.matmul(
                    ps[:, :],
                    lhsT=qT[hb:hb + DH, t * 128:(t + 1) * 128],
                    rhs=pT[hb:hb + DH, nk * 512:(nk + 1) * 512],
                    start=True, stop=True)
                pss.append(ps)
            nc.vector.tensor_copy(sts[0][:, nk * 512:(nk + 1) * 512],
                                  pss[0][:, :])
            nc.scalar.copy(sts[1][:, nk * 512:(nk + 1) * 512], pss[1][:, :])
            if nk == NQC - 1:
                for h in range(HPC):
                    nc.gpsimd.dma_start(
                        out=bass.AP(gs[h], t * 128 * GROWS,
                                    [[GROWS, 128], [1, GROWS]]),
                        in_=sts[h][:, :])
                del pos_state[t]

        def emit_chunk(c, pos_tiles):
            """shifted reads + content+shift-add+exp + A.V + out-proj for
            q chunk c (512 q rows). pos_tiles: next-chunk pos-score tiles
            to interleave into the 16 content kt slots."""
            q0 = c * 512
            quanta = [(t, nk) for t in pos_tiles for nk in range(NQC)]
            # shifted natural reads: [128q, L k] bf16, contiguous in G
            # (4KB per-partition runs -> fast M2S-concat path). The last
            # row depends on the next chunk's boundary pos tile, which the
            # interleave order writes first, so a single read per tile is
            # dependency-safe. h0 on sync, h1 on scalar (parallel HWDGE
            # queues).
            shs = {}
            for tl in range(TPC):
                qt = c * TPC + tl
                for h, eng in ((0, nc.sync), (1, nc.scalar)):
                    sh = sh_pool.tile([128, L], gdt, tag=f"sh{h}_{tl}",
                                      name=f"sh{h}_{tl}")
                    eng.dma_start(
                        out=sh[:, :],
                        in_=bass.AP(gs[h], (L - 1) + qt * 128 * L,
                                    [[L, 128], [1, L]]))
                    shs[(tl, h)] = sh

            # content transposed + shifted-transpose-accumulate + exp
            attns = []
            for h in range(HPC):
                at = attn_pool.tile([128, NQT, 512], bf16, tag=f"attn{h}",
                                    name=f"attn{h}")
                attns.append(at)
            for kt in range(NQT):
                if kt < len(quanta):
                    pos_quantum(*quanta[kt])
                pcs = []
                for h in range(HPC):
                    hb = h * DH
                    pc = ct_psum.tile([128, 512], f32, tag="ct", name="ct")
                    nc.tensor.matmul(
                        pc[:, :],
                        lhsT=kT[hb:hb + DH, kt * 128:(kt + 1) * 128],
                        rhs=qT[hb:hb + DH, q0:q0 + 512],
                        start=True, stop=False)
                    pcs.append(pc)
                for h in range(HPC):
                    for tl in range(TPC):
                        nc.tensor.matmul(
                            pcs[h][:, tl * 128:(tl + 1) * 128],
                            lhsT=shs[(tl, h)][:, kt * 128:(kt + 1) * 128],
                            rhs=id_sb[:, :],
                            start=False, stop=(tl == TPC - 1))
                for h in range(HPC):
                    nc.scalar.activation(
                        attns[h][:, kt, :], pcs[h][:, :],
                        AF.Exp, bias=0.0, scale=SCALE)

            # A.V (transposed): ctx_T [64, 512] + Z row via ones-col.
            # cx is a full [128, 512] bank; after it drains, the Z-row
            # transpose matmuls (Z values onto partitions) reuse it.
            ctxs = []
            zrows = []
            cxs = []
            for h in range(HPC):
                cx = cx_psum.tile([128, 512], f32, tag="cx", name="cx")
                for kt in range(NQT):
                    nc.tensor.matmul(
                        cx[0:DH + 1, :],
                        lhsT=vaug[:, kt, h * (DH + 1):(h + 1) * (DH + 1)],
                        rhs=attns[h][:, kt, :],
                        start=(kt == 0), stop=(kt == NQT - 1))
                ctx = ctx_pool.tile([DH, 512], bf16, tag=f"ctx{h}",
                                    name=f"ctx{h}")
                zrow = z_pool.tile([1, 512], f32, tag=f"z{h}", name=f"z{h}")
                nc.vector.tensor_copy(ctx[:, :], cx[0:DH, :])
                nc.scalar.copy(zrow[0:1, :], cx[DH:DH + 1, :])
                ctxs.append(ctx)
                zrows.append(zrow)
                cxs.append(cx)
            rzs = []
            for h in range(HPC):
                for tl in range(TPC):
                    nc.tensor.matmul(
                        cxs[h][:, tl:tl + 1],
                        lhsT=zrows[h][0:1, tl * 128:(tl + 1) * 128],
                        rhs=ones_sb[0:1, 0:1], start=True, stop=True)
                rz = rz_pool.tile([128, TPC], f32, tag=f"rz{h}",
                                  name=f"rz{h}")
                nc.vector.reciprocal(rz[:, :], cxs[h][:, 0:TPC])
                rzs.append(rz)

            # out projection + 1/Z normalization per 128-row tile
            for tl in range(TPC):
                po0 = po_psum.tile([128, 512], f32, tag="po")
                nc.tensor.matmul(po0[:, :],
                                 lhsT=ctxs[0][:, tl * 128:(tl + 1) * 128],
                                 rhs=wo_sb0[:, :], start=True, stop=True)
                po1 = po_psum.tile([128, 512], f32, tag="po")
                nc.tensor.matmul(po1[:, :],
                                 lhsT=ctxs[1][:, tl * 128:(tl + 1) * 128],
                                 rhs=wo_sb1[:, :], start=True, stop=True)
                tm = tmp_pool.tile([128, 512], f32, tag="tmp")
                nc.vector.tensor_scalar_mul(tm[:, :], po0[:, :],
                                            rzs[0][:, tl:tl + 1])
                ot = out_pool.tile([128, 512], bf16, tag="out")
                nc.vector.scalar_tensor_tensor(
                    ot[:, :], po1[:, :], rzs[1][:, tl:tl + 1], tm[:, :],
                    op0=ALU.mult, op1=ALU.add)
                trow = c * TPC + tl
                nc.scalar.dma_start(out=out[trow * 128:(trow + 1) * 128, :],
                                    in_=ot[:, :])

        # software pipeline: pos runs one chunk (+1 boundary tile) ahead of
        # the content/attention chunk, so the G write->read roundtrip
        # overlaps. Each chunk's next-next boundary tile goes first in its
        # interleave list so its G write lands before the next chunk needs
        # its row-127 reads.
        for t in range(TPC + 1):
            for nk in range(NQC):
                pos_quantum(t, nk)
        interleave = [[2 * TPC], [3 * TPC], [], []]
        for c in range(2):
            interleave[c] += list(range((c + 1) * TPC + 1, (c + 2) * TPC))
        interleave[2] = list(range(3 * TPC + 1, NQT))
        for c in range(NQC):
            emit_chunk(c, interleave[c])

    return nc


def _shard_inputs(query, key, value, pos_emb, Wq, bq, Wk, bk, Wv, bv, Wp, Wo, bo):
    """Build the 8 per-core input maps (host-side, free)."""
    in_maps = []
    xt = {}
    for b in range(B):
        xt[("q", b)] = np.ascontiguousarray(query[b].T).astype(_BF16)
        xt[("k", b)] = np.ascontiguousarray(key[b].T).astype(_BF16)
        xt[("p", b)] = np.ascontiguousarray(pos_emb[b].T).astype(_BF16)
        xt[("v", b)] = np.ascontiguousarray(value[b].T).astype(_BF16)
    wq16, wk16, wp16, wv16, wo16 = (w.astype(_BF16) for w in (Wq, Wk, Wp, Wv, Wo))
    ident = np.eye(128, dtype=np.float32).astype(_BF16)
    for c in range(NCORES):
        b, hp = c // 4, c % 4
        cs = slice(hp * HPC * DH, (hp + 1) * HPC * DH)
        in_maps.append({
            "xq_t": xt[("q", b)],
            "xk_t": xt[("k", b)],
            "xp_t": xt[("p", b)],
            "xv_t": xt[("v", b)],
            "ident": ident,
            "wq": np.ascontiguousarray(wq16[:, cs]),
            "wk": np.ascontiguousarray(wk16[:, cs]),
            "wp": np.ascontiguousarray(wp16[:, cs]),
            "wv": np.ascontiguousarray(wv16[:, cs]),
            "wo0": np.ascontiguousarray(wo16[hp * HPC * DH:hp * HPC * DH + DH, :]),
            "wo1": np.ascontiguousarray(wo16[hp * HPC * DH + DH:(hp + 1) * HPC * DH, :]),
            "bq": np.ascontiguousarray(bq[cs]).reshape(HPC * DH, 1).astype(np.float32),
            "bk": np.ascontiguousarray(bk[cs]).reshape(HPC * DH, 1).astype(np.float32),
        })
    return in_maps


def _unshard(results, Wo, bv, bo):
    const = (bv.astype(np.float32) @ Wo.astype(np.float32)) + bo.astype(np.float32)
    out = np.zeros((B, L, D), np.float32)
    for c in range(NCORES):
        out[c // 4] += results[c]["out"].astype(np.float32)
    out += const[None, None, :]
    return out


_CACHE = {}


def kernel(query, key, value, pos_emb, Wq, bq, Wk, bk, Wv, bv, Wp, Wo, bo,
           _want_profile=False):
    import sys
    if "/opt/trn_rl_repo" not in sys.path:
        sys.path.insert(0, "/opt/trn_rl_repo")
    from concourse.bass_utils import run_bass_kernel_spmd

    args = [np.asarray(a) for a in
            (query, key, value, pos_emb, Wq, bq, Wk, bk, Wv, bv, Wp, Wo, bo)]
    (query, key, value, pos_emb, Wq, bq, Wk, bk, Wv, bv, Wp, Wo, bo) = args

    if "nc" not in _CACHE:
        nc = build_nc()
        if not nc.is_finalized():
            nc.finalize()
        _CACHE["nc"] = nc
    nc = _CACHE["nc"]

    in_maps = _shard_inputs(query, key, value, pos_emb, Wq, bq, Wk, bk, Wv, bv,
                            Wp, Wo, bo)
    res = run_bass_kernel_spmd(nc, in_maps, list(range(NCORES)),
                               trace=_want_profile)
    out = _unshard(res.results, Wo, bv, bo)
    if _want_profile:
        return out, res
    return out


# revision 17
# speedup vs baseline: 1.6092x; 1.0767x over previous
"""Trainium2 Bass kernel for nn_RelativeMultiHeadAttention.

Full (unsharded) numpy inputs in, full output out. Internally shards across
8 NeuronCores: core c handles batch b = c//4 and head pair hp = c%4
(heads 2*hp, 2*hp+1).

Device pipeline (per core), fully "transposed" orientation:
  - host supplies query/key/value/pos_emb transposed ([D, L]) and bf16-cast,
    plus per-head-pair column slices of Wq/Wk/Wv/Wp and row slices of Wo.
  - projections on PE produce q_T/k_T/p_T ([2*dh, L], channels on partitions)
    and v ([L, 2*dh], natural), biases folded in via ACT bias adds.
  - pos scores S = q_h @ p_h^T computed natural ([lq, lk]); written to a
    DRAM buffer G padded to [L, L+1] rows (pad col = 0).
  - relative shift: shifted[q, k] = G_flat[q*L + (L-1) + k]  (Transformer-XL
    pad/reshape/slice trick becomes a strided read of the padded buffer).
    Read back TRANSPOSED via the DMA x-bar (bf16) => shifted_T [lk, lq].
  - content scores computed transposed ([lk, lq]); DVE adds shifted_T,
    ACT applies exp(scale * logit) -> attn_T (unnormalized, bf16).
    No max-subtraction needed: |logit*scale| < ~4 for these inputs.
  - A.V: lhsT = [v_h | ones] so PSUM row 64 accumulates Z = sum_k attn.
  - out projection per head (K=64) + per-partition 1/Z normalization, summed
    across the two heads on DVE; partial output [L, D] fp32 to DRAM.
Host sums the 4 per-core partials of each batch and adds bv @ Wo + bo
(exact: attention rows sum to 1 after normalization).
"""

import numpy as np
import ml_dtypes

B, L, D, H = 2, 2048, 512, 8
DH = D // H            # 64
HPC = 2                # heads per core
NCORES = 8
SCALE = 1.0 / float(np.sqrt(D))
LQT = L // 128         # 16 q/k tiles of 128
NQC = L // 512         # 4 chunks of 512
GROWS = L + 1          # padded G row length (2049)

_BF16 = ml_dtypes.bfloat16


def build_nc():
    import concourse.bass as bass
    import concourse.mybir as mybir
    from concourse.bacc import Bacc
    from concourse.tile import TileContext
    from contextlib import ExitStack

    bf16 = mybir.dt.bfloat16
    f32 = mybir.dt.float32
    AF = mybir.ActivationFunctionType
    ALU = mybir.AluOpType

    nc = Bacc()

    # ---- I/O ----
    xq = nc.declare_dram_parameter("xq_t", [D, L], bf16, isOutput=False)
    xk = nc.declare_dram_parameter("xk_t", [D, L], bf16, isOutput=False)
    xp = nc.declare_dram_parameter("xp_t", [D, L], bf16, isOutput=False)
    xv = nc.declare_dram_parameter("xv_t", [D, L], bf16, isOutput=False)
    wq = nc.declare_dram_parameter("wq", [D, HPC * DH], bf16, isOutput=False)
    wk = nc.declare_dram_parameter("wk", [D, HPC * DH], bf16, isOutput=False)
    wp = nc.declare_dram_parameter("wp", [D, HPC * DH], bf16, isOutput=False)
    wv = nc.declare_dram_parameter("wv", [D, HPC * DH], bf16, isOutput=False)
    wo0 = nc.declare_dram_parameter("wo0", [DH, D], bf16, isOutput=False)
    wo1 = nc.declare_dram_parameter("wo1", [DH, D], bf16, isOutput=False)
    ident = nc.declare_dram_parameter("ident", [128, 128], bf16, isOutput=False)
    bq = nc.declare_dram_parameter("bq", [HPC * DH, 1], f32, isOutput=False)
    bk = nc.declare_dram_parameter("bk", [HPC * DH, 1], f32, isOutput=False)
    out = nc.declare_dram_parameter("out", [L, D], f32, isOutput=True)

    # scratch DRAM for the relative-shift roundtrip, one per head
    g0 = nc.dram_tensor("g0", [L * GROWS], bf16)
    g1 = nc.dram_tensor("g1", [L * GROWS], bf16)
    gs = [g0, g1]

    with TileContext(nc) as tc, ExitStack() as top:
        # ---------- persistent SBUF (one pool, one tag per tensor) ----------
        persist = top.enter_context(tc.tile_pool(name="persist", bufs=1))

        def ptile(shape, dtype, name):
            return persist.tile(shape, dtype, name=name, tag=name)

        qT = ptile([128, L], bf16, "qT")
        kT = ptile([128, L], bf16, "kT")
        pT = ptile([128, L], bf16, "pT")
        vaug = ptile([128, LQT, 2 * (DH + 1)], bf16, "vaug")
        wo_sb0 = ptile([DH, D], bf16, "wo_sb0")
        wo_sb1 = ptile([DH, D], bf16, "wo_sb1")
        bq_sb = ptile([128, 1], f32, "bq_sb")
        bk_sb = ptile([128, 1], f32, "bk_sb")
        ones_sb = ptile([1, 1], f32, "ones_sb")
        id_sb = ptile([128, 128], bf16, "id_sb")

        nc.vector.memset(ones_sb[:, :], 1.0)
        nc.vector.memset(vaug[:, :, DH:DH + 1], 1.0)
        nc.vector.memset(vaug[:, :, 2 * DH + 1:2 * DH + 2], 1.0)
        nc.gpsimd.dma_start(out=id_sb[:, :], in_=ident[:, :])
        nc.gpsimd.dma_start(out=wo_sb0[:, :], in_=wo0[:, :])
        nc.gpsimd.dma_start(out=wo_sb1[:, :], in_=wo1[:, :])
        nc.gpsimd.dma_start(out=bq_sb[:, :], in_=bq[:, :])
        nc.gpsimd.dma_start(out=bk_sb[:, :], in_=bk[:, :])

        # ---------- phase 1: projections (inputs pool freed afterwards) ----
        with ExitStack() as p1:
            inpool = p1.enter_context(tc.tile_pool(name="inpool", bufs=1))
            x_sbs = {}
            w_sbs = {}
            for name, src in (("q", xq), ("k", xk), ("p", xp), ("v", xv)):
                t = inpool.tile([128, 4, L], bf16, name=f"x_{name}", tag=f"x_{name}")
                for c in range(4):
                    eng = nc.sync if c % 2 == 0 else nc.gpsimd
                    eng.dma_start(
                        out=t[:, c, :], in_=src[c * 128:(c + 1) * 128, :])
                x_sbs[name] = t
            for name, src in (("q", wq), ("k", wk), ("p", wp), ("v", wv)):
                t = inpool.tile([128, 4, HPC * DH], bf16, name=f"w_{name}",
                                tag=f"w_{name}")
                nc.gpsimd.dma_start(
                    out=t[:, :, :], in_=src[:, :].rearrange("(c p) m -> p c m", p=128)
                )
                w_sbs[name] = t

            pj_psum = p1.enter_context(
                tc.tile_pool(name="pj_psum", bufs=3, space="PSUM"))
            v_psum = p1.enter_context(
                tc.tile_pool(name="v_psum", bufs=2, space="PSUM"))

            # q_T / k_T / p_T : [128 (2 heads * 64 ch), L]
            for name, dst, bias in (("q", qT, bq_sb), ("k", kT, bk_sb),
                                    ("p", pT, None)):
                xs, ws = x_sbs[name], w_sbs[name]
                for n in range(NQC):
                    ps = pj_psum.tile([128, 512], f32, tag="pj")
                    for c in range(4):
                        nc.tensor.matmul(
                            ps[:, :], lhsT=ws[:, c, :],
                            rhs=xs[:, c, n * 512:(n + 1) * 512],
                            start=(c == 0), stop=(c == 3))
                    o = dst[:, n * 512:(n + 1) * 512]
                    if bias is not None:
                        nc.scalar.activation(o, ps[:, :], AF.Identity,
                                             bias=bias[:, 0:1], scale=1.0)
                    else:
                        nc.scalar.copy(o, ps[:, :])

            # v natural: [L, 128ch] -> vaug [128, t, [v0|1|v1|1]]
            xs, ws = x_sbs["v"], w_sbs["v"]
            for t in range(LQT):
                ps = v_psum.tile([128, 128], f32, tag="v")
                for c in range(4):
                    nc.tensor.matmul(
                        ps[:, :], lhsT=xs[:, c, t * 128:(t + 1) * 128],
                        rhs=ws[:, c, :], start=(c == 0), stop=(c == 3))
                nc.vector.tensor_copy(vaug[:, t, 0:DH], ps[:, 0:DH])
                nc.vector.tensor_copy(vaug[:, t, DH + 1:2 * DH + 1],
                                      ps[:, DH:2 * DH])

        # ---------- phase 2: scores / shift / softmax / A.V ------------
        attn_pool = top.enter_context(tc.tile_pool(name="attn_pool", bufs=1))
        attn0 = attn_pool.tile([128, LQT, L], bf16, name="attn0", tag="attn0")
        attn1 = attn_pool.tile([128, LQT, L], bf16, name="attn1", tag="attn1")
        attns = [attn0, attn1]

        with ExitStack() as p2:
            s_psum = p2.enter_context(
                tc.tile_pool(name="s_psum", bufs=4, space="PSUM"))
            s_stage = p2.enter_context(tc.tile_pool(name="s_stage", bufs=4))

            # --- pos scores S, natural [lq, lk], streamed to padded G.
            # Heads interleaved: h0 on PE row-group 0-1, h1 on 2-3 (K=64
            # matmuls execute concurrently in the array).
            for t in range(LQT):
                sts = []
                for h in range(HPC):
                    st = s_stage.tile([128, GROWS], bf16, tag=f"sstage{h}",
                                      name=f"st{h}")
                    nc.vector.memset(st[:, L:GROWS], 0.0)
                    sts.append(st)
                pss = {}
                for half in range(2):
                    for h in range(HPC):
                        hb = h * DH
                        ps = s_psum.tile([128, 1024], f32, tag="s", name="ps_s")
                        for qc in range(2):
                            n = half * 2 + qc
                            nc.tensor.matmul(
                                ps[:, qc * 512:(qc + 1) * 512],
                                lhsT=qT[hb:hb + DH, t * 128:(t + 1) * 128],
                                rhs=pT[hb:hb + DH, n * 512:(n + 1) * 512],
                                start=True, stop=True)
                        pss[(half, h)] = ps
                for half in range(2):
                    for h in range(HPC):
                        o = sts[h][:, half * 1024:(half + 1) * 1024]
                        if (half + h) % 2 == 0:
                            nc.scalar.copy(o, pss[(half, h)][:, :])
                        else:
                            nc.vector.tensor_copy(o, pss[(half, h)][:, :])
                for h in range(HPC):
                    nc.gpsimd.dma_start(
                        out=bass.AP(gs[h], t * 128 * GROWS,
                                    [[GROWS, 128], [1, GROWS]]),
                        in_=sts[h][:, :])

        with ExitStack() as p2b:
            ct_psum = p2b.enter_context(
                tc.tile_pool(name="ct_psum", bufs=4, space="PSUM"))
            sh_pool = p2b.enter_context(tc.tile_pool(name="sh_pool", bufs=3))
            lg_pool = p2b.enter_context(tc.tile_pool(name="lg_pool", bufs=2))

            # --- content_T + shifted_T -> exp -> attn_T (heads interleaved)
            for kt in range(LQT):
                shs = []
                for h in range(HPC):
                    sh = sh_pool.tile([128, L], bf16, tag=f"sh{h}",
                                      name=f"sh{h}")
                    nc.sync.dma_start_transpose(
                        out=sh[:, :],
                        in_=bass.AP(gs[h], (L - 1) + 128 * kt,
                                    [[L, L], [1, 128]]))
                    shs.append(sh)
                lgs = []
                for h in range(HPC):
                    lg = lg_pool.tile([128, L], f32, tag="lg", name="lg")
                    lgs.append(lg)
                for half in range(2):
                    cts = []
                    for h in range(HPC):
                        ct = ct_psum.tile([128, 1024], f32, tag="ct",
                                          name="ct")
                        cts.append(ct)
                    for qc in range(2):
                        for h in range(HPC):
                            hb = h * DH
                            q0 = half * 1024 + qc * 512
                            nc.tensor.matmul(
                                cts[h][:, qc * 512:(qc + 1) * 512],
                                lhsT=kT[hb:hb + DH, kt * 128:(kt + 1) * 128],
                                rhs=qT[hb:hb + DH, q0:q0 + 512],
                                start=True, stop=True)
                    for h in range(HPC):
                        nc.vector.tensor_add(
                            lgs[h][:, half * 1024:(half + 1) * 1024],
                            cts[h][:, :],
                            shs[h][:, half * 1024:(half + 1) * 1024])
                for h in range(HPC):
                    nc.scalar.activation(
                        attns[h][:, kt, :], lgs[h][:, :],
                        AF.Exp, bias=0.0, scale=SCALE)

        with ExitStack() as p2c:
            late = p2c.enter_context(tc.tile_pool(name="late", bufs=1))
            ctx0 = late.tile([DH, L], bf16, name="ctx0", tag="ctx0")
            ctx1 = late.tile([DH, L], bf16, name="ctx1", tag="ctx1")
            zrow0 = late.tile([1, L], f32, name="zrow0", tag="zrow0")
            zrow1 = late.tile([1, L], f32, name="zrow1", tag="zrow1")
            rz0 = late.tile([128, LQT], f32, name="rz0", tag="rz0")
            rz1 = late.tile([128, LQT], f32, name="rz1", tag="rz1")
            ctxs = [ctx0, ctx1]
            zrows = [zrow0, zrow1]
            rzs = [rz0, rz1]
            ctx_psum = p2c.enter_context(
                tc.tile_pool(name="ctx_psum", bufs=4, space="PSUM"))
            # --- A.V (transposed): ctx_T [64, L] + Z row, fused with the
            # output projection per query group so the tail overlaps ---
            z_psum = p2c.enter_context(
                tc.tile_pool(name="z_psum", bufs=1, space="PSUM"))
            o_psum = p2c.enter_context(
                tc.tile_pool(name="o_psum", bufs=2, space="PSUM"))
            tmp_pool = p2c.enter_context(tc.tile_pool(name="tmp_pool", bufs=2))
            out_pool = p2c.enter_context(tc.tile_pool(name="out_pool", bufs=3))
            for qg in range(NQC):
                cxs = []
                for h in range(HPC):
                    cx = ctx_psum.tile([DH + 1, 512], f32, tag="cx", name="cx")
                    cxs.append(cx)
                for kt in range(LQT):
                    for h in range(HPC):
                        nc.tensor.matmul(
                            cxs[h][:, :],
                            lhsT=vaug[:, kt, h * (DH + 1):(h + 1) * (DH + 1)],
                            rhs=attns[h][:, kt, qg * 512:(qg + 1) * 512],
                            start=(kt == 0), stop=(kt == LQT - 1))
                for h in range(HPC):
                    nc.vector.tensor_copy(
                        ctxs[h][:, qg * 512:(qg + 1) * 512], cxs[h][0:DH, :])
                    nc.scalar.copy(
                        zrows[h][0:1, qg * 512:(qg + 1) * 512],
                        cxs[h][DH:DH + 1, :])
                for t in range(qg * 4, (qg + 1) * 4):
                    for h in range(HPC):
                        zp = z_psum.tile([128, 1], f32, tag="z")
                        nc.tensor.matmul(
                            zp[:, :],
                            lhsT=zrows[h][0:1, t * 128:(t + 1) * 128],
                            rhs=ones_sb[0:1, 0:1], start=True, stop=True)
                        nc.vector.reciprocal(rzs[h][:, t:t + 1], zp[:, :])
                    po0 = o_psum.tile([128, 512], f32, tag="po")
                    nc.tensor.matmul(po0[:, :],
                                     lhsT=ctx0[:, t * 128:(t + 1) * 128],
                                     rhs=wo_sb0[:, :], start=True, stop=True)
                    po1 = o_psum.tile([128, 512], f32, tag="po")
                    nc.tensor.matmul(po1[:, :],
                                     lhsT=ctx1[:, t * 128:(t + 1) * 128],
                                     rhs=wo_sb1[:, :], start=True, stop=True)
                    tm = tmp_pool.tile([128, 512], f32, tag="tmp")
                    nc.vector.tensor_scalar_mul(tm[:, :], po0[:, :],
                                                rz0[:, t:t + 1])
                    ot = out_pool.tile([128, 512], f32, tag="out")
                    nc.vector.scalar_tensor_tensor(
                        ot[:, :], po1[:, :], rz1[:, t:t + 1], tm[:, :],
                        op0=ALU.mult, op1=ALU.add)
                    nc.gpsimd.dma_start(out=out[t * 128:(t + 1) * 128, :],
                                        in_=ot[:, :])

    return nc


def _shard_inputs(query, key, value, pos_emb, Wq, bq, Wk, bk, Wv, bv, Wp, Wo, bo):
    """Build the 8 per-core input maps (host-side, free)."""
    in_maps = []
    xt = {}
    for b in range(B):
        xt[("q", b)] = np.ascontiguousarray(query[b].T).astype(_BF16)
        xt[("k", b)] = np.ascontiguousarray(key[b].T).astype(_BF16)
        xt[("p", b)] = np.ascontiguousarray(pos_emb[b].T).astype(_BF16)
        xt[("v", b)] = np.ascontiguousarray(value[b].T).astype(_BF16)
    wq16, wk16, wp16, wv16, wo16 = (w.astype(_BF16) for w in (Wq, Wk, Wp, Wv, Wo))
    ident = np.eye(128, dtype=np.float32).astype(_BF16)
    for c in range(NCORES):
        b, hp = c // 4, c % 4
        cs = slice(hp * HPC * DH, (hp + 1) * HPC * DH)
        in_maps.append({
            "xq_t": xt[("q", b)],
            "xk_t": xt[("k", b)],
            "xp_t": xt[("p", b)],
            "xv_t": xt[("v", b)],
            "ident": ident,
            "wq": np.ascontiguousarray(wq16[:, cs]),
            "wk": np.ascontiguousarray(wk16[:, cs]),
            "wp": np.ascontiguousarray(wp16[:, cs]),
            "wv": np.ascontiguousarray(wv16[:, cs]),
            "wo0": np.ascontiguousarray(wo16[hp * HPC * DH:hp * HPC * DH + DH, :]),
            "wo1": np.ascontiguousarray(wo16[hp * HPC * DH + DH:(hp + 1) * HPC * DH, :]),
            "bq": np.ascontiguousarray(bq[cs]).reshape(HPC * DH, 1).astype(np.float32),
            "bk": np.ascontiguousarray(bk[cs]).reshape(HPC * DH, 1).astype(np.float32),
        })
    return in_maps


def _unshard(results, Wo, bv, bo):
    const = (bv.astype(np.float32) @ Wo.astype(np.float32)) + bo.astype(np.float32)
    out = np.zeros((B, L, D), np.float32)
    for c in range(NCORES):
        out[c // 4] += results[c]["out"].astype(np.float32)
    out += const[None, None, :]
    return out


_CACHE = {}


def kernel(query, key, value, pos_emb, Wq, bq, Wk, bk, Wv, bv, Wp, Wo, bo,
           _want_profile=False):
    import sys
    if "/opt/trn_rl_repo" not in sys.path:
        sys.path.insert(0, "/opt/trn_rl_repo")
    from concourse.bass_utils import run_bass_kernel_spmd

    args = [np.asarray(a) for a in
            (query, key, value, pos_emb, Wq, bq, Wk, bk, Wv, bv, Wp, Wo, bo)]
    (query, key, value, pos_emb, Wq, bq, Wk, bk, Wv, bv, Wp, Wo, bo) = args

    if "nc" not in _CACHE:
        nc = build_nc()
        if not nc.is_finalized():
            nc.finalize()
        _CACHE["nc"] = nc
    nc = _CACHE["nc"]

    in_maps = _shard_inputs(query, key, value, pos_emb, Wq, bq, Wk, bk, Wv, bv,
                            Wp, Wo, bo)
    res = run_bass_kernel_spmd(nc, in_maps, list(range(NCORES)),
                               trace=_want_profile)
    out = _unshard(res.results, Wo, bv, bo)
    if _want_profile:
        return out, res
    return out


if __name__ == "__main__":
    import jax
    jax.config.update("jax_platforms", "cpu")



# revision 24
# speedup vs baseline: 1.7978x; 1.1172x over previous
"""Trainium2 Bass kernel for nn_RelativeMultiHeadAttention.

Full (unsharded) numpy inputs in, full output out. Internally shards across
8 NeuronCores: core c handles batch b = c//4 and head pair hp = c%4
(heads 2*hp, 2*hp+1).

Device pipeline (per core), fully "transposed" orientation:
  - host supplies query/key/value/pos_emb transposed ([D, L]) and bf16-cast,
    plus per-head-pair column slices of Wq/Wk/Wv/Wp and row slices of Wo.
  - projections on PE produce q_T/k_T/p_T ([2*dh, L], channels on partitions)
    and v ([L, 2*dh], natural), biases folded in via ACT bias adds.
  - pos scores S = q_h @ p_h^T computed natural ([lq, lk]); written to a
    DRAM buffer G padded to [L, L+1] rows (pad col = 0).
  - relative shift: shifted[q, k] = G_flat[q*L + (L-1) + k]  (Transformer-XL
    pad/reshape/slice trick becomes a strided read of the padded buffer).
    Read back TRANSPOSED via the DMA x-bar (bf16) => shifted_T [lk, lq].
  - content scores computed transposed ([lk, lq]); DVE adds shifted_T,
    ACT applies exp(scale * logit) -> attn_T (unnormalized, bf16).
    No max-subtraction needed: |logit*scale| < ~4 for these inputs.
  - A.V: lhsT = [v_h | ones] so PSUM row 64 accumulates Z = sum_k attn.
  - out projection per head (K=64) + per-partition 1/Z normalization, summed
    across the two heads on DVE; partial output [L, D] fp32 to DRAM.
Host sums the 4 per-core partials of each batch and adds bv @ Wo + bo
(exact: attention rows sum to 1 after normalization).
"""

import numpy as np
import ml_dtypes

B, L, D, H = 2, 2048, 512, 8
DH = D // H            # 64
HPC = 2                # heads per core
NCORES = 8
SCALE = 1.0 / float(np.sqrt(D))
LQT = L // 128         # 16 q/k tiles of 128
NQC = L // 512         # 4 chunks of 512
GROWS = L + 1          # padded G row length (2049)

_BF16 = ml_dtypes.bfloat16


def build_nc():
    import concourse.bass as bass
    import concourse.mybir as mybir
    from concourse.bacc import Bacc
    from concourse.tile import TileContext
    from contextlib import ExitStack

    bf16 = mybir.dt.bfloat16
    f32 = mybir.dt.float32
    AF = mybir.ActivationFunctionType
    ALU = mybir.AluOpType

    nc = Bacc()

    # ---- I/O ----
    xq = nc.declare_dram_parameter("xq_t", [D, L], bf16, isOutput=False)
    xk = nc.declare_dram_parameter("xk_t", [D, L], bf16, isOutput=False)
    xp = nc.declare_dram_parameter("xp_t", [D, L], bf16, isOutput=False)
    xv = nc.declare_dram_parameter("xv_t", [D, L], bf16, isOutput=False)
    wq = nc.declare_dram_parameter("wq", [D, HPC * DH], bf16, isOutput=False)
    wk = nc.declare_dram_parameter("wk", [D, HPC * DH], bf16, isOutput=False)
    wp = nc.declare_dram_parameter("wp", [D, HPC * DH], bf16, isOutput=False)
    wv = nc.declare_dram_parameter("wv", [D, HPC * DH], bf16, isOutput=False)
    wo0 = nc.declare_dram_parameter("wo0", [DH, D], bf16, isOutput=False)
    wo1 = nc.declare_dram_parameter("wo1", [DH, D], bf16, isOutput=False)
    ident = nc.declare_dram_parameter("ident", [128, 128], bf16, isOutput=False)
    bq = nc.declare_dram_parameter("bq", [HPC * DH, 1], f32, isOutput=False)
    bk = nc.declare_dram_parameter("bk", [HPC * DH, 1], f32, isOutput=False)
    out = nc.declare_dram_parameter("out", [L, D], bf16, isOutput=True)

    # scratch DRAM for the relative-shift roundtrip, one per head
    g0 = nc.dram_tensor("g0", [L * GROWS], bf16)
    g1 = nc.dram_tensor("g1", [L * GROWS], bf16)
    gs = [g0, g1]

    with TileContext(nc) as tc, ExitStack() as top:
        # ---------- persistent SBUF (one pool, one tag per tensor) ----------
        persist = top.enter_context(tc.tile_pool(name="persist", bufs=1))

        def ptile(shape, dtype, name):
            return persist.tile(shape, dtype, name=name, tag=name)

        qT = ptile([128, L], bf16, "qT")
        kT = ptile([128, L], bf16, "kT")
        pT = ptile([128, L], bf16, "pT")
        vaug = ptile([128, LQT, 2 * (DH + 1)], bf16, "vaug")
        wo_sb0 = ptile([DH, D], bf16, "wo_sb0")
        wo_sb1 = ptile([DH, D], bf16, "wo_sb1")
        bq_sb = ptile([128, 1], f32, "bq_sb")
        bk_sb = ptile([128, 1], f32, "bk_sb")
        ones_sb = ptile([1, 1], f32, "ones_sb")
        id_sb = ptile([128, 128], bf16, "id_sb")

        nc.vector.memset(ones_sb[:, :], 1.0)
        nc.vector.memset(vaug[:, :, DH:DH + 1], 1.0)
        nc.vector.memset(vaug[:, :, 2 * DH + 1:2 * DH + 2], 1.0)
        nc.gpsimd.dma_start(out=id_sb[:, :], in_=ident[:, :])
        nc.gpsimd.dma_start(out=wo_sb0[:, :], in_=wo0[:, :])
        nc.gpsimd.dma_start(out=wo_sb1[:, :], in_=wo1[:, :])
        nc.gpsimd.dma_start(out=bq_sb[:, :], in_=bq[:, :])
        nc.gpsimd.dma_start(out=bk_sb[:, :], in_=bk[:, :])

        # ---------- phase 1: projections (inputs pool freed afterwards) ----
        with ExitStack() as p1:
            inpool = p1.enter_context(tc.tile_pool(name="inpool", bufs=1))
            x_sbs = {}
            w_sbs = {}
            for name, src in (("q", xq), ("k", xk), ("p", xp), ("v", xv)):
                t = inpool.tile([128, 4, L], bf16, name=f"x_{name}", tag=f"x_{name}")
                for c in range(4):
                    eng = (nc.sync, nc.gpsimd, nc.scalar)[c % 3]
                    eng.dma_start(
                        out=t[:, c, :], in_=src[c * 128:(c + 1) * 128, :])
                x_sbs[name] = t
            for name, src in (("q", wq), ("k", wk), ("p", wp), ("v", wv)):
                t = inpool.tile([128, 4, HPC * DH], bf16, name=f"w_{name}",
                                tag=f"w_{name}")
                nc.gpsimd.dma_start(
                    out=t[:, :, :], in_=src[:, :].rearrange("(c p) m -> p c m", p=128)
                )
                w_sbs[name] = t

            pj_psum = p1.enter_context(
                tc.tile_pool(name="pj_psum", bufs=3, space="PSUM"))
            v_psum = p1.enter_context(
                tc.tile_pool(name="v_psum", bufs=2, space="PSUM"))

            # q_T / k_T / p_T : [128 (2 heads * 64 ch), L]
            for name, dst, bias in (("q", qT, bq_sb), ("k", kT, bk_sb),
                                    ("p", pT, None)):
                xs, ws = x_sbs[name], w_sbs[name]
                for n in range(NQC):
                    ps = pj_psum.tile([128, 512], f32, tag="pj")
                    for c in range(4):
                        nc.tensor.matmul(
                            ps[:, :], lhsT=ws[:, c, :],
                            rhs=xs[:, c, n * 512:(n + 1) * 512],
                            start=(c == 0), stop=(c == 3))
                    o = dst[:, n * 512:(n + 1) * 512]
                    if bias is not None:
                        nc.scalar.activation(o, ps[:, :], AF.Identity,
                                             bias=bias[:, 0:1], scale=1.0)
                    else:
                        nc.scalar.copy(o, ps[:, :])

            # v natural: [L, 128ch] -> vaug [128, t, [v0|1|v1|1]]
            xs, ws = x_sbs["v"], w_sbs["v"]
            for t in range(LQT):
                ps = v_psum.tile([128, 128], f32, tag="v")
                for c in range(4):
                    nc.tensor.matmul(
                        ps[:, :], lhsT=xs[:, c, t * 128:(t + 1) * 128],
                        rhs=ws[:, c, :], start=(c == 0), stop=(c == 3))
                nc.vector.tensor_copy(vaug[:, t, 0:DH], ps[:, 0:DH])
                nc.vector.tensor_copy(vaug[:, t, DH + 1:2 * DH + 1],
                                      ps[:, DH:2 * DH])

        # ---------- phase 2: scores / shift / softmax / A.V ------------
        attn_pool = top.enter_context(tc.tile_pool(name="attn_pool", bufs=1))
        attn0 = attn_pool.tile([128, LQT, L], bf16, name="attn0", tag="attn0")
        attn1 = attn_pool.tile([128, LQT, L], bf16, name="attn1", tag="attn1")
        attns = [attn0, attn1]

        with ExitStack() as p2:
            s_psum = p2.enter_context(
                tc.tile_pool(name="s_psum", bufs=4, space="PSUM"))
            s_stage = p2.enter_context(tc.tile_pool(name="s_stage", bufs=4))

            # --- pos scores S, natural [lq, lk], streamed to padded G.
            # Heads interleaved: h0 on PE row-group 0-1, h1 on 2-3 (K=64
            # matmuls execute concurrently in the array).
            for t in range(LQT):
                sts = []
                for h in range(HPC):
                    st = s_stage.tile([128, GROWS], bf16, tag=f"sstage{h}",
                                      name=f"st{h}")
                    nc.vector.memset(st[:, L:GROWS], 0.0)
                    sts.append(st)
                pss = {}
                for half in range(2):
                    for h in range(HPC):
                        hb = h * DH
                        ps = s_psum.tile([128, 1024], f32, tag="s", name="ps_s")
                        for qc in range(2):
                            n = half * 2 + qc
                            nc.tensor.matmul(
                                ps[:, qc * 512:(qc + 1) * 512],
                                lhsT=qT[hb:hb + DH, t * 128:(t + 1) * 128],
                                rhs=pT[hb:hb + DH, n * 512:(n + 1) * 512],
                                start=True, stop=True)
                        pss[(half, h)] = ps
                for half in range(2):
                    for h in range(HPC):
                        o = sts[h][:, half * 1024:(half + 1) * 1024]
                        if (half + h) % 2 == 0:
                            nc.scalar.copy(o, pss[(half, h)][:, :])
                        else:
                            nc.vector.tensor_copy(o, pss[(half, h)][:, :])
                # h0 writes on gpsimd (SWDGE), h1 on sync (idle until the
                # phase-2b reads, which sit behind the pool-close barrier)
                for h, eng in ((0, nc.gpsimd), (1, nc.sync)):
                    eng.dma_start(
                        out=bass.AP(gs[h], t * 128 * GROWS,
                                    [[GROWS, 128], [1, GROWS]]),
                        in_=sts[h][:, :])

        with ExitStack() as p2b:
            ct_psum = p2b.enter_context(
                tc.tile_pool(name="ct_psum", bufs=4, space="PSUM"))
            sh_pool = p2b.enter_context(tc.tile_pool(name="sh_pool", bufs=3))
            lg_pool = p2b.enter_context(tc.tile_pool(name="lg_pool", bufs=2))

            # --- content_T + shifted_T -> exp -> attn_T (heads interleaved)
            for kt in range(LQT):
                shs = []
                for h in range(HPC):
                    sh = sh_pool.tile([128, L], bf16, tag=f"sh{h}",
                                      name=f"sh{h}")
                    nc.sync.dma_start_transpose(
                        out=sh[:, :],
                        in_=bass.AP(gs[h], (L - 1) + 128 * kt,
                                    [[L, L], [1, 128]]))
                    shs.append(sh)
                lgs = []
                for h in range(HPC):
                    lg = lg_pool.tile([128, L], f32, tag="lg", name="lg")
                    lgs.append(lg)
                for half in range(2):
                    cts = []
                    for h in range(HPC):
                        ct = ct_psum.tile([128, 1024], f32, tag="ct",
                                          name="ct")
                        cts.append(ct)
                    for qc in range(2):
                        for h in range(HPC):
                            hb = h * DH
                            q0 = half * 1024 + qc * 512
                            nc.tensor.matmul(
                                cts[h][:, qc * 512:(qc + 1) * 512],
                                lhsT=kT[hb:hb + DH, kt * 128:(kt + 1) * 128],
                                rhs=qT[hb:hb + DH, q0:q0 + 512],
                                start=True, stop=True)
                    for h in range(HPC):
                        nc.vector.tensor_add(
                            lgs[h][:, half * 1024:(half + 1) * 1024],
                            cts[h][:, :],
                            shs[h][:, half * 1024:(half + 1) * 1024])
                for h in range(HPC):
                    nc.scalar.activation(
                        attns[h][:, kt, :], lgs[h][:, :],
                        AF.Exp, bias=0.0, scale=SCALE)

        with ExitStack() as p2c:
            late = p2c.enter_context(tc.tile_pool(name="late", bufs=1))
            ctx0 = late.tile([DH, L], bf16, name="ctx0", tag="ctx0")
            ctx1 = late.tile([DH, L], bf16, name="ctx1", tag="ctx1")
            zrow0 = late.tile([1, L], f32, name="zrow0", tag="zrow0")
            zrow1 = late.tile([1, L], f32, name="zrow1", tag="zrow1")
            rz0 = late.tile([128, LQT], f32, name="rz0", tag="rz0")
            rz1 = late.tile([128, LQT], f32, name="rz1", tag="rz1")
            ctxs = [ctx0, ctx1]
            zrows = [zrow0, zrow1]
            rzs = [rz0, rz1]
            ctx_psum = p2c.enter_context(
                tc.tile_pool(name="ctx_psum", bufs=4, space="PSUM"))
            # --- A.V (transposed): ctx_T [64, L] + Z row, fused with the
            # output projection per query group so the tail overlaps ---
            z_psum = p2c.enter_context(
                tc.tile_pool(name="z_psum", bufs=1, space="PSUM"))
            o_psum = p2c.enter_context(
                tc.tile_pool(name="o_psum", bufs=2, space="PSUM"))
            tmp_pool = p2c.enter_context(tc.tile_pool(name="tmp_pool", bufs=2))
            out_pool = p2c.enter_context(tc.tile_pool(name="out_pool", bufs=3))
            for qg in range(NQC):
                cxs = []
                for h in range(HPC):
                    cx = ctx_psum.tile([DH + 1, 512], f32, tag="cx", name="cx")
                    cxs.append(cx)
                for kt in range(LQT):
                    for h in range(HPC):
                        nc.tensor.matmul(
                            cxs[h][:, :],
                            lhsT=vaug[:, kt, h * (DH + 1):(h + 1) * (DH + 1)],
                            rhs=attns[h][:, kt, qg * 512:(qg + 1) * 512],
                            start=(kt == 0), stop=(kt == LQT - 1))
                for h in range(HPC):
                    nc.vector.tensor_copy(
                        ctxs[h][:, qg * 512:(qg + 1) * 512], cxs[h][0:DH, :])
                    nc.scalar.copy(
                        zrows[h][0:1, qg * 512:(qg + 1) * 512],
                        cxs[h][DH:DH + 1, :])
                for t in range(qg * 4, (qg + 1) * 4):
                    for h in range(HPC):
                        zp = z_psum.tile([128, 1], f32, tag="z")
                        nc.tensor.matmul(
                            zp[:, :],
                            lhsT=zrows[h][0:1, t * 128:(t + 1) * 128],
                            rhs=ones_sb[0:1, 0:1], start=True, stop=True)
                        nc.vector.reciprocal(rzs[h][:, t:t + 1], zp[:, :])
                    po0 = o_psum.tile([128, 512], f32, tag="po")
                    nc.tensor.matmul(po0[:, :],
                                     lhsT=ctx0[:, t * 128:(t + 1) * 128],
                                     rhs=wo_sb0[:, :], start=True, stop=True)
                    po1 = o_psum.tile([128, 512], f32, tag="po")
                    nc.tensor.matmul(po1[:, :],
                                     lhsT=ctx1[:, t * 128:(t + 1) * 128],
                                     rhs=wo_sb1[:, :], start=True, stop=True)
                    tm = tmp_pool.tile([128, 512], f32, tag="tmp")
                    nc.vector.tensor_scalar_mul(tm[:, :], po0[:, :],
                                                rz0[:, t:t + 1])
                    ot = out_pool.tile([128, 512], bf16, tag="out")
                    nc.vector.scalar_tensor_tensor(
                        ot[:, :], po1[:, :], rz1[:, t:t + 1], tm[:, :],
                        op0=ALU.mult, op1=ALU.add)
                    nc.gpsimd.dma_start(out=out[t * 128:(t + 1) * 128, :],
                                        in_=ot[:, :])

    return nc


def _shard_inputs(query, key, value, pos_emb, Wq, bq, Wk, bk, Wv, bv, Wp, Wo, bo):
    """Build the 8 per-core input maps (host-side, free)."""
    in_maps = []
    xt = {}
    for b in range(B):
        xt[("q", b)] = np.ascontiguousarray(query[b].T).astype(_BF16)
        xt[("k", b)] = np.ascontiguousarray(key[b].T).astype(_BF16)
        xt[("p", b)] = np.ascontiguousarray(pos_emb[b].T).astype(_BF16)
        xt[("v", b)] = np.ascontiguousarray(value[b].T).astype(_BF16)
    wq16, wk16, wp16, wv16, wo16 = (w.astype(_BF16) for w in (Wq, Wk, Wp, Wv, Wo))
    ident = np.eye(128, dtype=np.float32).astype(_BF16)
    for c in range(NCORES):
        b, hp = c // 4, c % 4
        cs = slice(hp * HPC * DH, (hp + 1) * HPC * DH)
        in_maps.append({
            "xq_t": xt[("q", b)],
            "xk_t": xt[("k", b)],
            "xp_t": xt[("p", b)],
            "xv_t": xt[("v", b)],
            "ident": ident,
            "wq": np.ascontiguousarray(wq16[:, cs]),
            "wk": np.ascontiguousarray(wk16[:, cs]),
            "wp": np.ascontiguousarray(wp16[:, cs]),
            "wv": np.ascontiguousarray(wv16[:, cs]),
            "wo0": np.ascontiguousarray(wo16[hp * HPC * DH:hp * HPC * DH + DH, :]),
            "wo1": np.ascontiguousarray(wo16[hp * HPC * DH + DH:(hp + 1) * HPC * DH, :]),
            "bq": np.ascontiguousarray(bq[cs]).reshape(HPC * DH, 1).astype(np.float32),
            "bk": np.ascontiguousarray(bk[cs]).reshape(HPC * DH, 1).astype(np.float32),
        })
    return in_maps


def _unshard(results, Wo, bv, bo):
    const = (bv.astype(np.float32) @ Wo.astype(np.float32)) + bo.astype(np.float32)
    out = np.zeros((B, L, D), np.float32)
    for c in range(NCORES):
        out[c // 4] += results[c]["out"].astype(np.float32)
    out += const[None, None, :]
    return out


_CACHE = {}


def kernel(query, key, value, pos_emb, Wq, bq, Wk, bk, Wv, bv, Wp, Wo, bo,
           _want_profile=False):
    import sys
    if "/opt/trn_rl_repo" not in sys.path:
        sys.path.insert(0, "/opt/trn_rl_repo")
    from concourse.bass_utils import run_bass_kernel_spmd

    args = [np.asarray(a) for a in
            (query, key, value, pos_emb, Wq, bq, Wk, bk, Wv, bv, Wp, Wo, bo)]
    (query, key, value, pos_emb, Wq, bq, Wk, bk, Wv, bv, Wp, Wo, bo) = args

    if "nc" not in _CACHE:
        nc = build_nc()
        if not nc.is_finalized():
            nc.finalize()
        _CACHE["nc"] = nc
    nc = _CACHE["nc"]

    in_maps = _shard_inputs(query, key, value, pos_emb, Wq, bq, Wk, bk, Wv, bv,
                            Wp, Wo, bo)
    res = run_bass_kernel_spmd(nc, in_maps, list(range(NCORES)),
                               trace=_want_profile)
    out = _unshard(res.results, Wo, bv, bo)
    if _want_profile:
        return out, res
    return out


if __name__ == "__main__":
    import jax
    jax.config.update("jax_platforms", "cpu")

